# revision 1
# baseline (speedup 1.0000x reference)
"""Additive-attention kernel for 8 TRN2 NeuronCores.

reference:
    x = concat([s, h], axis=1)            # (N, 2D)
    X = tanh(x @ W.T)                     # (N, 2*DA)
    pre = (X @ v.T).T                     # (1, N)
    out = softmax(pre, axis=1)            # (1, N)

Strategy: shard rows (N) across 8 cores (4096 rows each). W, v replicated.
Each core computes tanh(x_shard @ W.T) @ v.T fused in SBUF/PSUM (bf16
matmul, fp32 accumulate), produces 4096 scores, takes exp, sums locally,
AllReduces the per-core sums, and normalizes its shard by the global sum.
Softmax max-subtraction is skipped: |score| <= ||v||_1 ~ 33 << 88 (fp32 exp
overflow), so exp is always finite and the result is exact to fp32.

The matmul phase runs at the power-throttled PE roofline: under
sustained 8-core load the PE is clock-limited to ~1.95-2.0 GHz (either
the board GPIO throttler gating to K=13/16 of 2.4 GHz, or the P0 power
state), giving ~263 ns per 128x128x512 bf16 matmul; measured PE idle
within the span is <2%. fp8 DoubleRow (2x PE rate, verified 216 ns for
K=256 on this HW) was evaluated and rejected: e4m3 quantization of both
operands yields 3.1e-2 final error (vs the 2e-2 gate, measured against
the seeded reference), and any residual-corrected scheme costs as many
PE cycles as bf16. The structure minimizes everything around the PE:
 - a short warm-up matmul run on the first arrived x slice keeps the
   HAM activity clock-gate open through the DMA-bound head.
 - W streams as full k-tiles in consumption order; x prefetch and the
   v replica are issued behind it (W for all 8 cores is 67 MB of HBM
   reads - the head is HBM-bound, so W owns the early window).
 - row-tiles 0 and 1 interleave over k so each arriving W k-tile feeds
   8 matmuls; later tiles run k-outer / j-inner (4 consecutive matmuls
   share the stationary x-tile; redundant LDWEIGHTS stripped
   post-build). The last tile is chunk-major so its drain pipelines.
 - tail: one Exp with fused accumulate, partition reduce, a single
   4-byte AllReduce(add) of the 8 partial sums, broadcast, scale,
   store. ~5 us from last matmul to collective trigger.

Host-side prep is layout only (transpose/concat/cast + replicate v).
"""

import numpy as np
import ml_dtypes

N, D, DA = 32768, 1024, 1024
NCORES = 8
NS = N // NCORES            # 4096 rows per core
P = 128
MT = NS // P                # 32 row-tiles per core
KIN = 2 * D                 # 2048 contraction
KT = KIN // P               # 16 k-tiles
NOUT = 2 * DA               # 2048 out features
NCH = 512                   # psum chunk (one bank of fp32)
NCK = NOUT // NCH           # 4 chunks


def _build_nc():
    from concourse import bacc, mybir, tile, bass

    f32 = mybir.dt.float32
    bf16 = mybir.dt.bfloat16
    AF = mybir.ActivationFunctionType
    ALU = mybir.AluOpType
    AX = mybir.AxisListType

    nc = bacc.Bacc(
        "TRN2",
        target_bir_lowering=False,
        debug=False,
        num_devices=NCORES,
    )

    xh = nc.declare_dram_parameter("xh", [NS, KIN], bf16, isOutput=False)
    wt = nc.declare_dram_parameter("wt", [KIN, NOUT], bf16, isOutput=False)
    vr = nc.declare_dram_parameter("vr", [P, NOUT], f32, isOutput=False)
    out_ext = nc.declare_dram_parameter("out", [P, MT], f32, isOutput=True)

    with tile.TileContext(nc) as tc:
        with (
            tc.tile_pool(name="wpool", bufs=1) as wpool,
            tc.tile_pool(name="xpool", bufs=4) as xpool,
            tc.tile_pool(name="tpool", bufs=3) as tpool,
            tc.tile_pool(name="spool", bufs=1) as spool,
            tc.tile_pool(name="ppool", bufs=2, space="PSUM") as ppool,
            tc.tile_pool(name="dpool", bufs=1, space="DRAM") as dpool,
        ):
            # first x k-slice, then W tiles in k (consumption) order.
            # W owns the DMA rings early: x prefetch and v are issued behind
            # the W descriptors so W tiles complete as early as possible.
            xm0 = xpool.tile([P, KIN], bf16, name="xm", tag="xm")
            nc.sync.dma_start(out=xm0[:, 0:P], in_=xh[0:P, 0:P])
            wsb = [
                wpool.tile([P, NOUT], bf16, name=f"wk{k}") for k in range(KT)
            ]
            # w0 in halves: the first real matmuls (k0, j0/j1) only wait on
            # the first 1024 columns, starting ~1.5us earlier.
            # ALL W rides the sync queue in strict k order (x/v go on
            # scalar): W rows then hit the rings in exactly consumption
            # order, instead of interleaving across two queues' ring
            # assignments and completing lumpily.
            nc.sync.dma_start(out=wsb[0][:, 0:1024], in_=wt[0:P, 0:1024])
            nc.scalar.dma_start(out=xm0[:, P:1024], in_=xh[0:P, P:1024])
            nc.sync.dma_start(out=wsb[0][:, 1024:NOUT], in_=wt[0:P, 1024:NOUT])
            nc.scalar.dma_start(out=xm0[:, 1024:KIN], in_=xh[0:P, 1024:KIN])
            for k in range(1, KT // 2):
                nc.sync.dma_start(
                    out=wsb[k][:, :], in_=wt[k * P:(k + 1) * P, :]
                )

            # prioritize W k0..7 on the DMA rings: the rings round-robin all
            # queued rows, so issuing all 16 tiles at once makes early
            # k-tiles complete as late as the last ones and stalls the
            # in-order PE queue. This tiny SBUF->DRAM dma stalls the sync
            # queue until k7 lands, so k8..15 only hit the rings afterwards
            # (consumption of k8..15 starts ~6us later than that).
            wh_gate = dpool.tile([1, 1], bf16, name="wh_gate")
            nc.sync.dma_start(out=wh_gate[:, :], in_=wsb[KT // 2 - 1][0:1, 0:1])
            for k in range(KT // 2, KT):
                nc.sync.dma_start(
                    out=wsb[k][:, :], in_=wt[k * P:(k + 1) * P, :]
                )

            # PE pre-warm on the first x slice (lands ~1.5us in): keeps the
            # PE busy so the HAM activity clock-gate opens before real work;
            # results land in a psum bank that the real stream later resets
            pswarm = ppool.tile([P, NCH], f32, name="ps0", tag="ps0")
            for _ in range(22):
                nc.tensor.matmul(
                    pswarm[:, 0:P], lhsT=xm0[:, 0:P], rhs=xm0[:, 0:P],
                    start=True, stop=True,
                )

            def load_xm(m, eng):
                t = xpool.tile([P, KIN], bf16, name="xm", tag="xm")
                eng.dma_start(out=t[:, :], in_=xh[m * P:(m + 1) * P, :])
                return t

            xm_pre = [xm0, load_xm(1, nc.scalar)]

            # rendezvous the 8 cores while the weight DMAs stream in, so the
            # tail collective doesn't pay launch-skew latency
            sync_in = dpool.tile([1, 1], f32, name="sync_in")
            sync_out = dpool.tile(
                [1, NCORES], f32, name="sync_out", addr_space="Shared"
            )
            nc.gpsimd.collective_compute(
                "AllGather",
                ALU.bypass,
                replica_groups=[list(range(NCORES))],
                ins=[sync_in.opt()],
                outs=[sync_out.opt()],
            )
            # v replica loads on scalar with the x traffic (needed ~45us in)
            vsb = wpool.tile([P, NOUT], f32, name="vsb")
            nc.scalar.dma_start(out=vsb[:, :], in_=vr[:, :])

            # gate the early x prefetches behind W completion: this copy
            # stalls the gpsimd queue until the last W tile lands, so the
            # prefetch DMAs it issues next can't steal ring bandwidth from
            # the W stream (their deadline is ~48us+)
            wgate = spool.tile([1, 1], bf16, name="wgate")
            nc.gpsimd.tensor_copy(wgate[0:1, 0:1], wsb[KT - 1][0:1, 0:1])

            scores = spool.tile([P, MT], f32, name="scores")
            expv = spool.tile([P, MT], f32, name="expv")
            zrow = spool.tile([P, 1], f32, name="zrow")

            def alloc_work(m):
                psums = []
                for j in range(NCK):
                    ps = ppool.tile([P, NCH], f32, name=f"ps{j}", tag=f"ps{j}")
                    psums.append(ps)
                tmt = tpool.tile([P, NOUT], f32, name="tmt", tag="tmt")
                umt = tpool.tile([P, NOUT], f32, name="umt", tag="umt")
                acc = tpool.tile([P, NCK], f32, name="acc", tag="acc")
                return psums, tmt, umt, acc

            def drain(m, psums, tmt, umt, acc, j):
                sl = slice(j * NCH, (j + 1) * NCH)
                nc.scalar.activation(tmt[:, sl], psums[j][:, :], AF.Tanh)
                # one DVE op: umt = tanh*v, acc[:,j] = row-sum(umt)
                nc.vector.scalar_tensor_tensor(
                    out=umt[:, sl],
                    in0=tmt[:, sl],
                    scalar=1.0,
                    in1=vsb[:, sl],
                    op0=ALU.mult,
                    op1=ALU.mult,
                    accum_out=acc[:, j:j + 1],
                )

            def finish_scores(m, acc):
                nc.vector.tensor_reduce(
                    scores[:, m:m + 1], acc[:, :], AX.X, ALU.add
                )
                # incremental exp per tile: the softmax tail then only
                # needs a small reduce over expv instead of a bulk Exp
                nc.scalar.activation(
                    expv[:, m:m + 1], scores[:, m:m + 1], AF.Exp
                )

            # tiles 0 and 1 interleaved over k: 8 matmuls per arriving W
            # k-tile keep the PE saturated while W streams in (8.4 MB takes
            # ~25us; a single tile only holds 17us of work)
            work01 = [alloc_work(0), alloc_work(1)]
            for k in range(KT):
                for m in (0, 1):
                    for j in range(NCK):
                        nc.tensor.matmul(
                            work01[m][0][j][:, :],
                            lhsT=xm_pre[m][:, k * P:(k + 1) * P],
                            rhs=wsb[k][:, j * NCH:(j + 1) * NCH],
                            start=(k == 0),
                            stop=(k == KT - 1),
                        )
            for m in (0, 1):
                psums, tmt, umt, acc = work01[m]
                for j in range(NCK):
                    drain(m, psums, tmt, umt, acc, j)
                finish_scores(m, acc)

            for m in range(2, MT):
                # early prefetches go on the gpsimd queue behind the W-gate
                # copy; later ones are gated by xpool instance reuse anyway
                if m < 10:
                    eng = nc.gpsimd
                else:
                    eng = nc.sync if m % 2 == 0 else nc.gpsimd
                xm = load_xm(m, eng)
                psums, tmt, umt, acc = alloc_work(m)

                if m < MT - 1:
                    # k-outer: the 4 matmuls per k share the stationary x
                    # tile (LDWEIGHTS dedup below)
                    for k in range(KT):
                        for j in range(NCK):
                            nc.tensor.matmul(
                                psums[j][:, :],
                                lhsT=xm[:, k * P:(k + 1) * P],
                                rhs=wsb[k][:, j * NCH:(j + 1) * NCH],
                                start=(k == 0),
                                stop=(k == KT - 1),
                            )
                    for j in range(NCK):
                        drain(m, psums, tmt, umt, acc, j)
                    finish_scores(m, acc)
                else:
                    # last tile chunk-major so each chunk drains while the
                    # next chunk's matmuls run, in shrinking pieces (halves,
                    # then quarters for the final chunk) so the very last
                    # tanh+mul on the critical chain is a quarter-chunk:
                    # shortens every core's path to the collective trigger
                    acc10 = tpool.tile(
                        [P, 2 * NCK + 1], f32, name="acc10", tag="acc10"
                    )
                    NH = NCH // 2
                    NQ = NCH // 4
                    ac = 0
                    for j in range(NCK):
                        for k in range(KT):
                            nc.tensor.matmul(
                                psums[j][:, :],
                                lhsT=xm[:, k * P:(k + 1) * P],
                                rhs=wsb[k][:, j * NCH:(j + 1) * NCH],
                                start=(k == 0),
                                stop=(k == KT - 1),
                            )
                        widths = [NH, NH] if j < NCK - 1 else [NH, NQ, NQ]
                        off = 0
                        for w in widths:
                            sl = slice(j * NCH + off, j * NCH + off + w)
                            psl = slice(off, off + w)
                            nc.scalar.activation(
                                tmt[:, sl], psums[j][:, psl], AF.Tanh
                            )
                            nc.vector.scalar_tensor_tensor(
                                out=umt[:, sl],
                                in0=tmt[:, sl],
                                scalar=1.0,
                                in1=vsb[:, sl],
                                op0=ALU.mult,
                                op1=ALU.mult,
                                accum_out=acc10[:, ac:ac + 1],
                            )
                            off += w
                            ac += 1
                    nc.vector.tensor_reduce(
                        scores[:, m:m + 1], acc10[:, :], AX.X, ALU.add
                    )
                    nc.scalar.activation(
                        expv[:, m:m + 1], scores[:, m:m + 1], AF.Exp
                    )

            # ---- softmax over the global N via one AllGather ----
            # expv is already filled per tile; sum it, then an in-SBUF
            # partition all-reduce (faster than tensor_reduce over AX.C)
            from concourse import bass_isa

            nc.vector.tensor_reduce(
                zrow[:, 0:1], expv[:, :], AX.X, ALU.add
            )
            zloc = spool.tile([P, 1], f32, name="zloc")
            nc.gpsimd.partition_all_reduce(
                zloc[:, 0:1], zrow[:, 0:1], channels=P,
                reduce_op=bass_isa.ReduceOp.add,
            )
            # AllGather instead of AllReduce: the gather is a 2-phase mesh
            # op vs AllReduce's 4 phases (~2-3us cheaper after the last
            # contribution); the 8-way add costs one ~80ns DVE op instead
            zin = dpool.tile([1, 1], f32, name="zin")
            zout = dpool.tile(
                [1, NCORES], f32, name="zout", addr_space="Shared"
            )
            nc.gpsimd.dma_start(out=zin[:, :], in_=zloc[0:1, 0:1])
            nc.gpsimd.collective_compute(
                "AllGather",
                ALU.bypass,
                replica_groups=[list(range(NCORES))],
                ins=[zin.opt()],
                outs=[zout.opt()],
            )
            # broadcast the 8 partials to every partition (stride-0 DRAM
            # read), reduce + reciprocal once, then scale the shard, store
            zgb = spool.tile([P, NCORES], f32, name="zgb")
            zout_bc = bass.AP(
                zout.tensor, zout.offset, [(0, P), (1, NCORES)]
            )
            nc.gpsimd.dma_start(out=zgb[:, :], in_=zout_bc)
            zp = spool.tile([P, 1], f32, name="zp")
            nc.vector.tensor_reduce(zp[:, 0:1], zgb[:, :], AX.X, ALU.add)
            rzb = spool.tile([P, 1], f32, name="rzb")
            nc.vector.reciprocal(rzb[:, 0:1], zp[:, 0:1])
            # scale+store in halves on separate queues: the second half's
            # multiply overlaps the first half's output DMA
            outsb = spool.tile([P, MT], f32, name="outsb")
            MH = MT // 2
            nc.vector.tensor_scalar_mul(
                outsb[:, 0:MH], expv[:, 0:MH], rzb[:, 0:1]
            )
            nc.sync.dma_start(out=out_ext[:, 0:MH], in_=outsb[:, 0:MH])
            nc.vector.tensor_scalar_mul(
                outsb[:, MH:MT], expv[:, MH:MT], rzb[:, 0:1]
            )
            nc.scalar.dma_start(out=out_ext[:, MH:MT], in_=outsb[:, MH:MT])

    # run_bass_via_pjrt binds the exec primitive directly and skips the
    # finalize that bass_jit flows do; Bacc register allocation runs here.
    nc.finalize()
    _strip_redundant_ldweights(nc)
    return nc


def _strip_redundant_ldweights(nc):
    """Bacc's move_matmul_waits_to_ldweights emits one InstLdweights per
    matmul even when consecutive matmuls share the stationary operand.
    The PE keeps the loaded weights across matmuls, so an Ldweights whose
    weights AP equals the previous one's and that carries no semaphore
    waits/updates is pure redundant load time (~110ns each on the PE
    critical path). Drop them; only the matmuls (ldweights=false) remain."""
    def sig(arg):
        return (
            getattr(arg, "memref", None),
            getattr(arg, "offset", None),
            str(getattr(arg, "ap", None)),
        )

    removed = 0
    for bb in nc.main_func.blocks:
        keep = []
        last = None
        for inst in bb.instructions:
            if "Ldweights" in type(inst).__name__:
                s = sig(inst.ins[0])
                si = inst.sync_info
                if s == last and (
                    si is None or (not si.on_wait and not si.on_update)
                ):
                    removed += 1
                    continue
                last = s
            keep.append(inst)
        bb.instructions = keep
    return removed


def _prep_core_inputs(s, h, W, v):
    """Host-side layout prep: per-core tiled x^T, shared W^T, replicated v."""
    bf16 = ml_dtypes.bfloat16
    wt = np.ascontiguousarray(W.T).astype(bf16)          # [KIN, NOUT]
    vrep = np.ascontiguousarray(
        np.broadcast_to(v.reshape(1, NOUT), (P, NOUT))
    ).astype(np.float32)

    in_maps = []
    for c in range(NCORES):
        sl = slice(c * NS, (c + 1) * NS)
        x = np.concatenate([s[sl], h[sl]], axis=1)       # [NS, KIN]
        # xh[m*128+kk, k*128+rr] = x[m*128+rr, k*128+kk]
        xh = (
            x.reshape(MT, P, KT, P)
            .transpose(0, 3, 2, 1)
            .reshape(NS, KIN)
        )
        xh = np.ascontiguousarray(xh).astype(bf16)
        in_maps.append({"xh": xh, "wt": wt, "vr": vrep})
    return in_maps


_RUN_KW = {}  # test.py can inject trace=True etc.
LAST_RESULT = None


def kernel(s, h, W, v):
    from concourse.bass_utils import run_bass_kernel_spmd

    global LAST_RESULT
    s = np.asarray(s, dtype=np.float32)
    h = np.asarray(h, dtype=np.float32)
    W = np.asarray(W, dtype=np.float32)
    v = np.asarray(v, dtype=np.float32)

    in_maps = _prep_core_inputs(s, h, W, v)
    res = None
    for attempt in range(3):
        nc = _build_nc()
        try:
            res = run_bass_kernel_spmd(
                nc, in_maps, core_ids=list(range(NCORES)), **_RUN_KW
            )
            break
        except Exception:
            # transient NRT_EXEC_UNIT_UNRECOVERABLE states clear on the
            # next attempt; rebuild and retry
            if attempt == 2:
                raise
            import time
            time.sleep(15)
    LAST_RESULT = res

    outs = []
    for c in range(NCORES):
        oc = np.asarray(res.results[c]["out"], dtype=np.float32)  # [P, MT]
        outs.append(oc.T.reshape(-1))                              # rows m*128+p
    return np.concatenate(outs).reshape(1, N).astype(np.float32)



# revision 3
# speedup vs baseline: 1.6709x; 1.6709x over previous
"""Additive-attention kernel for 8 TRN2 NeuronCores — fp8 DoubleRow hybrid.

reference:
    x = concat([s, h], axis=1)            # (N, 2D)
    X = tanh(x @ W.T)                     # (N, 2*DA)
    pre = (X @ v.T).T                     # (1, N)
    out = softmax(pre, axis=1)            # (1, N)

Rows (N) sharded across 8 cores (4096 each); W, v replicated. Per core
the (4096 x 2048) @ (2048 x 2048) GEMM runs 14 of 16 k-tiles in fp8
e4m3 with perf_mode=DoubleRow (2 k-tiles contracted per 512-cycle pass,
2x the bf16 MAC rate; verified 217 ns/pass on this HW) and the last 2
k-tiles in bf16, all accumulating into the same fp32 psum. bf16
operands are pre-scaled by 256 so every product carries the same 2^16
scale as the fp8 ones; the drain folds 2^-16 into the Tanh activation.

fp8 error management (the 2e-2 gate): plain e4m3 on both operands
measures 3.1e-2. Two measures bring it to ~1.6e-2:
 - error-shaped rounding (host, at quantization time): per W-row k the
   e4m3 rounding directions are chosen so sum_n v[n]*eps_W[k,n] ~= 0,
   and per x-row m so sum_k eps_x[m,k]*h[k] ~= 0 (h = W^T v restricted
   to the fp8 k-range). This cancels the mean-field first-order score
   error exactly; flips are picked by a damage/benefit greedy that
   prefers near-boundary elements, so the residual fluctuation term
   (tanh' variance weighted) is not inflated (<5% extra eps energy).
 - the last 2 k-tiles stay bf16, cutting the remaining fluctuation
   variance by 2/16.
uint8 matmul (3x lower quant error, ISA-documented with DoubleRow) was
tried and is dead on this toolchain: walrus codegen's cayman assert set
rejects non-fp8 dtypes for matmul/ldweights; e3m4 is likewise fp8-DR
excluded (s3*_dual_fp8_restrictions allows EXP4/EXP5 only).

Structure around the PE (inherited from the bf16 baseline, which ran at
the throttled ~1.95 GHz PE roofline with <2% idle):
 - warm-up matmuls on the first arrived x slice keep the HAM activity
   clock-gate open through the DMA-bound head.
 - W streams as 7 fp8 pair-tiles (4 KB/partition each) then 2 bf16
   k-tiles, in consumption order on the sync queue; a tiny gate DMA
   after pair 3 keeps early pairs prioritized on the rings. x prefetch
   and v ride the scalar/gpsimd queues behind a W-completion gate.
 - row-tiles 0 and 1 interleave over the pair stream (8 matmuls per
   arriving W tile); later tiles run pair-outer / chunk-inner so 4
   matmuls share each stationary LDWEIGHTS (redundant loads stripped
   post-build). The last tile is chunk-major with shrinking drain
   pieces to shorten the path to the collective.
 - tail: per-tile Exp with fused accumulate, partition reduce, one
   4-byte AllGather of the 8 partial sums, broadcast, scale, store.

Host-side prep: layout (transpose/concat/cast, v replication) plus the
quantization itself (scaling, shaped rounding). All O(N*K) elementwise;
the GEMM math happens on device.
"""

import numpy as np
import ml_dtypes

N, D, DA = 32768, 1024, 1024
NCORES = 8
NS = N // NCORES            # 4096 rows per core
P = 128
MT = NS // P                # 32 row-tiles per core
KIN = 2 * D                 # 2048 contraction
KT8 = 14                    # k-tiles in fp8 (7 DoubleRow pairs)
NPAIR = KT8 // 2
KTB = 2                     # k-tiles in bf16
KC = KT8 * P                # 1792 fp8 contraction columns
NOUT = 2 * DA               # 2048 out features
NCH = 512                   # psum chunk (one bank of fp32)
NCK = NOUT // NCH           # 4 chunks

SX = 32.0                   # fp8 x scale
SW = 2048.0                 # fp8 W scale
SB = 256.0                  # bf16 operand scale (SB*SB == SX*SW)
DESCALE = 1.0 / 65536.0

F8 = ml_dtypes.float8_e4m3
BF16 = ml_dtypes.bfloat16


def _build_nc():
    from concourse import bacc, mybir, tile, bass

    f32 = mybir.dt.float32
    bf16 = mybir.dt.bfloat16
    f8 = mybir.dt.float8e4
    AF = mybir.ActivationFunctionType
    ALU = mybir.AluOpType
    AX = mybir.AxisListType
    DR = mybir.MatmulPerfMode.DoubleRow

    nc = bacc.Bacc(
        "TRN2",
        target_bir_lowering=False,
        debug=False,
        num_devices=NCORES,
    )

    xh8 = nc.declare_dram_parameter("xh8", [NS, KC], f8, isOutput=False)
    xhb = nc.declare_dram_parameter("xhb", [NS, KTB * P], bf16, isOutput=False)
    w8 = nc.declare_dram_parameter("w8", [NPAIR * P, 2 * NOUT], f8,
                                   isOutput=False)
    wb = nc.declare_dram_parameter("wb", [KTB * P, NOUT], bf16,
                                   isOutput=False)
    vr = nc.declare_dram_parameter("vr", [P, NOUT], f32, isOutput=False)
    out_ext = nc.declare_dram_parameter("out", [P, MT], f32, isOutput=True)

    with tile.TileContext(nc) as tc:
        with (
            tc.tile_pool(name="wpool", bufs=1) as wpool,
            tc.tile_pool(name="xpool", bufs=4) as xpool,
            tc.tile_pool(name="tpool", bufs=3) as tpool,
            tc.tile_pool(name="spool", bufs=1) as spool,
            tc.tile_pool(name="ppool", bufs=2, space="PSUM") as ppool,
            tc.tile_pool(name="dpool", bufs=1, space="DRAM") as dpool,
        ):
            # first x k-slice for the PE warm-up, then W pair-tiles in
            # consumption order on the sync queue (x/v ride scalar).
            xm8_0 = xpool.tile([P, KT8, P], f8, name="xm8", tag="xm8")
            xmb_0 = xpool.tile([P, KTB, P], bf16, name="xmb", tag="xmb")
            nc.sync.dma_start(out=xm8_0[:, 0, :], in_=xh8[0:P, 0:P])
            w8sb = [
                wpool.tile([P, 2, NOUT], f8, name=f"w8p{t}")
                for t in range(NPAIR)
            ]
            wbsb = [
                wpool.tile([P, NOUT], bf16, name=f"wbk{i}")
                for i in range(KTB)
            ]
            nc.sync.dma_start(out=w8sb[0][:, 0, :], in_=w8[0:P, 0:NOUT])
            nc.scalar.dma_start(out=xm8_0[:, 1:KT8, :],
                                in_=xh8[0:P, P:KC])
            nc.sync.dma_start(out=w8sb[0][:, 1, :], in_=w8[0:P, NOUT:2 * NOUT])
            nc.scalar.dma_start(out=xmb_0[:, :, :], in_=xhb[0:P, :])
            for t in range(1, NPAIR // 2 + 1):
                nc.sync.dma_start(
                    out=w8sb[t][:, :, :], in_=w8[t * P:(t + 1) * P, :]
                )

            # prioritize the first half of the W stream on the rings: a
            # tiny SBUF->DRAM dma stalls the sync queue until pair 3
            # lands, so later tiles only hit the rings afterwards.
            wh_gate = dpool.tile([1, 1], f8, name="wh_gate")
            nc.sync.dma_start(out=wh_gate[:, :],
                              in_=w8sb[NPAIR // 2][0:1, 0, 0:1])
            for t in range(NPAIR // 2 + 1, NPAIR):
                nc.sync.dma_start(
                    out=w8sb[t][:, :, :], in_=w8[t * P:(t + 1) * P, :]
                )
            for i in range(KTB):
                nc.sync.dma_start(
                    out=wbsb[i][:, :], in_=wb[i * P:(i + 1) * P, :]
                )

            # PE pre-warm on the first x slice: keeps the HAM activity
            # clock-gate open before real work; plain fp8 matmuls.
            pswarm = ppool.tile([P, NCH], f32, name="ps0", tag="ps0")
            for _ in range(22):
                nc.tensor.matmul(
                    pswarm[:, 0:P], lhsT=xm8_0[:, 0, :], rhs=xm8_0[:, 0, :],
                    start=True, stop=True,
                )

            def load_xm(m, eng):
                t8 = xpool.tile([P, KT8, P], f8, name="xm8", tag="xm8")
                tb = xpool.tile([P, KTB, P], bf16, name="xmb", tag="xmb")
                eng.dma_start(out=t8[:, :, :], in_=xh8[m * P:(m + 1) * P, :])
                eng.dma_start(out=tb[:, :, :], in_=xhb[m * P:(m + 1) * P, :])
                return t8, tb

            xm_pre = [(xm8_0, xmb_0), load_xm(1, nc.scalar)]

            # rendezvous the 8 cores while the weight DMAs stream in
            sync_in = dpool.tile([1, 1], f32, name="sync_in")
            sync_out = dpool.tile(
                [1, NCORES], f32, name="sync_out", addr_space="Shared"
            )
            nc.gpsimd.collective_compute(
                "AllGather",
                ALU.bypass,
                replica_groups=[list(range(NCORES))],
                ins=[sync_in.opt()],
                outs=[sync_out.opt()],
            )
            vsb = wpool.tile([P, NOUT], f32, name="vsb")
            nc.scalar.dma_start(out=vsb[:, :], in_=vr[:, :])

            # gate the early x prefetches behind W completion
            wgate = spool.tile([1, 1], bf16, name="wgate")
            nc.gpsimd.tensor_copy(wgate[0:1, 0:1], wbsb[KTB - 1][0:1, 0:1])

            scores = spool.tile([P, MT], f32, name="scores")
            expv = spool.tile([P, MT], f32, name="expv")
            zrow = spool.tile([P, 1], f32, name="zrow")

            def alloc_work(m):
                psums = []
                for j in range(NCK):
                    ps = ppool.tile([P, NCH], f32, name=f"ps{j}", tag=f"ps{j}")
                    psums.append(ps)
                tmt = tpool.tile([P, NOUT], f32, name="tmt", tag="tmt")
                umt = tpool.tile([P, NOUT], f32, name="umt", tag="umt")
                acc = tpool.tile([P, NCK], f32, name="acc", tag="acc")
                return psums, tmt, umt, acc

            def mm_pair(psum, x8, t, j, start):
                nc.tensor.matmul(
                    psum[:, :],
                    lhsT=x8[:, 2 * t:2 * t + 2, :],
                    rhs=w8sb[t][:, :, j * NCH:(j + 1) * NCH],
                    start=start, stop=False,
                    perf_mode=DR,
                )

            def mm_bf(psum, xb, i, j, stop):
                nc.tensor.matmul(
                    psum[:, :],
                    lhsT=xb[:, i, :],
                    rhs=wbsb[i][:, j * NCH:(j + 1) * NCH],
                    start=False, stop=stop,
                )

            def drain(m, psums, tmt, umt, acc, j):
                sl = slice(j * NCH, (j + 1) * NCH)
                nc.scalar.activation(tmt[:, sl], psums[j][:, :], AF.Tanh,
                                     scale=DESCALE)
                nc.vector.scalar_tensor_tensor(
                    out=umt[:, sl],
                    in0=tmt[:, sl],
                    scalar=1.0,
                    in1=vsb[:, sl],
                    op0=ALU.mult,
                    op1=ALU.mult,
                    accum_out=acc[:, j:j + 1],
                )

            def finish_scores(m, acc):
                nc.vector.tensor_reduce(
                    scores[:, m:m + 1], acc[:, :], AX.X, ALU.add
                )
                nc.scalar.activation(
                    expv[:, m:m + 1], scores[:, m:m + 1], AF.Exp
                )

            # tiles 0 and 1 interleaved over the W stream: 8 matmuls per
            # arriving pair-tile keep the PE saturated while W lands
            work01 = [alloc_work(0), alloc_work(1)]
            for t in range(NPAIR):
                for m in (0, 1):
                    for j in range(NCK):
                        mm_pair(work01[m][0][j], xm_pre[m][0], t, j,
                                start=(t == 0))
            for i in range(KTB):
                for m in (0, 1):
                    for j in range(NCK):
                        mm_bf(work01[m][0][j], xm_pre[m][1], i, j,
                              stop=(i == KTB - 1))
            for m in (0, 1):
                psums, tmt, umt, acc = work01[m]
                for j in range(NCK):
                    drain(m, psums, tmt, umt, acc, j)
                finish_scores(m, acc)

            for m in range(2, MT):
                if m < 10:
                    eng = nc.gpsimd
                else:
                    eng = nc.sync if m % 2 == 0 else nc.gpsimd
                xm8, xmb = load_xm(m, eng)
                psums, tmt, umt, acc = alloc_work(m)

                if m < MT - 1:
                    for t in range(NPAIR):
                        for j in range(NCK):
                            mm_pair(psums[j], xm8, t, j, start=(t == 0))
                    for i in range(KTB):
                        for j in range(NCK):
                            mm_bf(psums[j], xmb, i, j, stop=(i == KTB - 1))
                    for j in range(NCK):
                        drain(m, psums, tmt, umt, acc, j)
                    finish_scores(m, acc)
                else:
                    # last tile: chunk-major so each chunk drains while
                    # the next chunk's matmuls run, shrinking pieces
                    acc10 = tpool.tile(
                        [P, 2 * NCK + 1], f32, name="acc10", tag="acc10"
                    )
                    NH = NCH // 2
                    NQ = NCH // 4
                    ac = 0
                    for j in range(NCK):
                        for t in range(NPAIR):
                            mm_pair(psums[j], xm8, t, j, start=(t == 0))
                        for i in range(KTB):
                            mm_bf(psums[j], xmb, i, j, stop=(i == KTB - 1))
                        widths = [NH, NH] if j < NCK - 1 else [NH, NQ, NQ]
                        off = 0
                        for w in widths:
                            sl = slice(j * NCH + off, j * NCH + off + w)
                            psl = slice(off, off + w)
                            nc.scalar.activation(
                                tmt[:, sl], psums[j][:, psl], AF.Tanh,
                                scale=DESCALE,
                            )
                            nc.vector.scalar_tensor_tensor(
                                out=umt[:, sl],
                                in0=tmt[:, sl],
                                scalar=1.0,
                                in1=vsb[:, sl],
                                op0=ALU.mult,
                                op1=ALU.mult,
                                accum_out=acc10[:, ac:ac + 1],
                            )
                            off += w
                            ac += 1
                    nc.vector.tensor_reduce(
                        scores[:, m:m + 1], acc10[:, :], AX.X, ALU.add
                    )
                    nc.scalar.activation(
                        expv[:, m:m + 1], scores[:, m:m + 1], AF.Exp
                    )

            # ---- softmax over the global N via one AllGather ----
            from concourse import bass_isa

            nc.vector.tensor_reduce(
                zrow[:, 0:1], expv[:, :], AX.X, ALU.add
            )
            zloc = spool.tile([P, 1], f32, name="zloc")
            nc.gpsimd.partition_all_reduce(
                zloc[:, 0:1], zrow[:, 0:1], channels=P,
                reduce_op=bass_isa.ReduceOp.add,
            )
            zin = dpool.tile([1, 1], f32, name="zin")
            zout = dpool.tile(
                [1, NCORES], f32, name="zout", addr_space="Shared"
            )
            nc.gpsimd.dma_start(out=zin[:, :], in_=zloc[0:1, 0:1])
            nc.gpsimd.collective_compute(
                "AllGather",
                ALU.bypass,
                replica_groups=[list(range(NCORES))],
                ins=[zin.opt()],
                outs=[zout.opt()],
            )
            zgb = spool.tile([P, NCORES], f32, name="zgb")
            zout_bc = bass.AP(
                zout.tensor, zout.offset, [(0, P), (1, NCORES)]
            )
            nc.gpsimd.dma_start(out=zgb[:, :], in_=zout_bc)
            zp = spool.tile([P, 1], f32, name="zp")
            nc.vector.tensor_reduce(zp[:, 0:1], zgb[:, :], AX.X, ALU.add)
            rzb = spool.tile([P, 1], f32, name="rzb")
            nc.vector.reciprocal(rzb[:, 0:1], zp[:, 0:1])
            outsb = spool.tile([P, MT], f32, name="outsb")
            MH = MT // 2
            nc.vector.tensor_scalar_mul(
                outsb[:, 0:MH], expv[:, 0:MH], rzb[:, 0:1]
            )
            nc.sync.dma_start(out=out_ext[:, 0:MH], in_=outsb[:, 0:MH])
            nc.vector.tensor_scalar_mul(
                outsb[:, MH:MT], expv[:, MH:MT], rzb[:, 0:1]
            )
            nc.scalar.dma_start(out=out_ext[:, MH:MT], in_=outsb[:, MH:MT])

    nc.finalize()
    _strip_redundant_ldweights(nc)
    return nc


def _strip_redundant_ldweights(nc):
    """Bacc emits one InstLdweights per matmul even when consecutive
    matmuls share the stationary operand. Drop the redundant ones."""
    def sig(arg):
        return (
            getattr(arg, "memref", None),
            getattr(arg, "offset", None),
            str(getattr(arg, "ap", None)),
        )

    removed = 0
    for bb in nc.main_func.blocks:
        keep = []
        last = None
        for inst in bb.instructions:
            if "Ldweights" in type(inst).__name__:
                s = sig(inst.ins[0])
                si = inst.sync_info
                if s == last and (
                    si is None or (not si.on_wait and not si.on_update)
                ):
                    removed += 1
                    continue
                last = s
            keep.append(inst)
        bb.instructions = keep
    return removed


# ---------------- host-side quantization ----------------

def _shaped_round(A, wcol, nscan):
    """e4m3 rounding of A (already scaled) with per-row flip choices so
    sum_j wcol[j]*eps[i,j] ~= 0, flips picked by damage/benefit greedy
    (near-boundary elements first) to keep eps energy ~unchanged.

    The alternative rounding (fp8 neighbor on the other side of A) is
    computed with sign-magnitude bit arithmetic on the e4m3 encoding."""
    A = np.clip(np.asarray(A, dtype=np.float32), -240.0, 240.0)
    q = A.astype(F8)
    qf = q.astype(np.float32)
    eps = qf - A
    u = q.view(np.uint8)
    sgn = u & np.uint8(0x80)
    mag = u & np.uint8(0x7F)
    pos = sgn == 0
    toward_pos = eps < 0          # q < A: the other neighbor is above q
    away = toward_pos == pos      # step increases |value|
    newmag = np.where(away, mag + np.uint8(1), mag - np.uint8(1))
    newsgn = sgn.copy()
    cross = (mag == 0) & ~away    # +/-0 stepping across zero
    newsgn = np.where(cross, sgn ^ np.uint8(0x80), newsgn)
    newmag = np.where(cross, np.uint8(1), newmag)
    alt = (newsgn | newmag).astype(np.uint8).view(F8).astype(np.float32)
    alt = np.where(eps == 0, qf, alt)
    epsa = alt - A
    w32 = wcol.astype(np.float32)
    c = (epsa - eps) * w32
    damage = (w32 ** 2) * (epsa ** 2 - eps ** 2)
    with np.errstate(divide="ignore", invalid="ignore"):
        ratio = np.where(np.abs(c) > 0, damage / np.abs(c), np.inf)
    nscan = min(nscan, A.shape[1] - 1)
    part = np.argpartition(ratio, nscan, axis=1)[:, :nscan]
    subr = np.take_along_axis(ratio, part, axis=1)
    subo = np.argsort(subr, axis=1)
    order = np.take_along_axis(part, subo, axis=1)
    c_s = np.take_along_axis(c, order, axis=1)
    g = (eps.astype(np.float64) @ wcol.astype(np.float64)).astype(np.float32)
    t = -g.copy()
    flip_s = np.zeros((A.shape[0], nscan), dtype=bool)
    for _ in range(2):
        for step in range(nscan):
            cn = np.where(flip_s[:, step], 0.0, c_s[:, step])
            take = np.abs(t - cn) < np.abs(t)
            t += np.where(take, -cn, 0.0)
            flip_s[:, step] |= take
    flip = np.zeros(A.shape, dtype=bool)
    np.put_along_axis(flip, order, flip_s, axis=1)
    return np.where(flip, alt, q).astype(F8)


def _prep_core_inputs(s, h, W, v):
    """Quantize + lay out per-core inputs."""
    x = np.concatenate([s, h], axis=1)                   # [N, KIN] f32
    Wt = np.ascontiguousarray(W.T)                       # [KIN, NOUT]
    vv = v.reshape(-1).astype(np.float64)
    hvec = Wt.astype(np.float64) @ vv                    # [KIN]

    W8 = _shaped_round(Wt[:KC].astype(np.float64) * SW, vv, nscan=512)
    X8 = _shaped_round(x[:, :KC].astype(np.float64) * SX, hvec[:KC],
                       nscan=384)

    # W pair tiles: w8[t*128+kk, i*NOUT+n] = W8[(2t+i)*128+kk, n]
    w8v = (
        W8.reshape(NPAIR, 2, P, NOUT)
        .transpose(0, 2, 1, 3)
        .reshape(NPAIR * P, 2 * NOUT)
    )
    w8v = np.ascontiguousarray(w8v)
    wbv = np.ascontiguousarray(Wt[KC:] * SB).astype(BF16)   # [256, NOUT]
    vrep = np.ascontiguousarray(
        np.broadcast_to(v.reshape(1, NOUT), (P, NOUT))
    ).astype(np.float32)

    xbv = (x[:, KC:] * SB).astype(BF16)                     # [N, 256]

    in_maps = []
    for c in range(NCORES):
        sl = slice(c * NS, (c + 1) * NS)
        x8c = X8[sl]                                        # [NS, KC]
        xh8 = (
            x8c.reshape(MT, P, KT8, P)
            .transpose(0, 3, 2, 1)
            .reshape(NS, KC)
        )
        xh8 = np.ascontiguousarray(xh8)
        xbc = xbv[sl]
        xhb = (
            xbc.reshape(MT, P, KTB, P)
            .transpose(0, 3, 2, 1)
            .reshape(NS, KTB * P)
        )
        xhb = np.ascontiguousarray(xhb)
        in_maps.append(
            {"xh8": xh8, "xhb": xhb, "w8": w8v, "wb": wbv, "vr": vrep}
        )
    return in_maps


_RUN_KW = {}  # test.py can inject trace=True etc.
LAST_RESULT = None


def kernel(s, h, W, v):
    from concourse.bass_utils import run_bass_kernel_spmd

    global LAST_RESULT
    s = np.asarray(s, dtype=np.float32)
    h = np.asarray(h, dtype=np.float32)
    W = np.asarray(W, dtype=np.float32)
    v = np.asarray(v, dtype=np.float32)

    in_maps = _prep_core_inputs(s, h, W, v)
    res = None
    for attempt in range(3):
        nc = _build_nc()
        try:
            res = run_bass_kernel_spmd(
                nc, in_maps, core_ids=list(range(NCORES)), **_RUN_KW
            )
            break
        except Exception:
            if attempt == 2:
                raise
            import time
            time.sleep(15)
    LAST_RESULT = res

    outs = []
    for c in range(NCORES):
        oc = np.asarray(res.results[c]["out"], dtype=np.float32)  # [P, MT]
        outs.append(oc.T.reshape(-1))
    return np.concatenate(outs).reshape(1, N).astype(np.float32)
